# revision 74
# baseline (speedup 1.0000x reference)
"""TRN2 Bass kernel for the two-encoder attention module.

Per batch element b (8 of them, one per NeuronCore):
    P1 = X @ E1^T          (T,S)   attention logits vs `sent`
    A1 = softmax(P1)               -> output sent_weight
    C1 = A1 @ E1           (T,H)
    P2/A2/C2 vs `template` (St)
    gate = sigmoid(cat(C1, C2, X) @ W_gate^T)        (T,H)
    fusion = tanh((1-gate)*C1@Ws^T + gate*C2@Wt^T + X@Wo^T)

Everything on-device is computed in a feature-major ("transposed") layout so
that every matmul contraction dim lands on SBUF partitions with no on-device
input transposes:
    - logits: lhsT = X^T (hi/lo bf16 split, 3-pass for fp32-grade accuracy),
      rhs = E^T (hi/lo).  Softmax along the free dim.
    - A^T obtained with PE transpose-mode matmuls (f16).
    - C^T  = E^T(natural lhsT) @ A^T    [f16]
    - gate^T, F^T via host-pretransposed/prepacked f16 weights.
Host side handles batch sharding across 8 cores, input transposes/casts and
the final un-transpose of fusion.
"""
import contextlib
import os

import ml_dtypes
import numpy as np

B, T, S, St, H = 8, 1024, 1024, 512, 1024
TB = T // 128           # 8 t-tiles
HB = H // 128           # 8 h-blocks
SB = S // 128           # 8 s-blocks (sent)
S2B = St // 128         # 4 s-blocks (template)
NT = T // 512           # 2 t-halves for 512-wide rhs

_CACHE = {}


def _build(reps=1, bc="f16", logits="hilo"):
    import concourse.bacc as bacc
    import concourse.mybir as mybir
    import concourse.tile as tile

    dt = mybir.dt
    F32, F16, BF16, F8 = dt.float32, dt.float16, dt.bfloat16, dt.float8e4
    if bc == "bf16":
        F16 = BF16  # dtype for the value/gate/fusion stages
    LG = dt.float32r if logits == "f32r" else BF16
    AF = mybir.ActivationFunctionType
    ALU = mybir.AluOpType
    AX = mybir.AxisListType

    nc = bacc.Bacc("TRN2", target_bir_lowering=False, debug=False, num_devices=8)

    d = {}
    decls = [
        ("xt_hi", [H, T], LG),
        ("e1t_hi", [H, S], LG),
        ("e2t_hi", [H, St], LG),
        ("e1f", [S, H], F16), ("e2f", [St, H], F16), ("xtf", [H, T], F16),
        ("wg", [H, 3 * H], F16), ("ws", [H, H], F16),
        ("wt", [H, H], F16), ("wo", [H, H], F16),
        ("ident", [128, 128], F16),
    ]
    if logits == "hilo":
        decls += [("xt_lo", [H, T], BF16), ("e1t_lo", [H, S], BF16),
                  ("e2t_lo", [H, St], BF16)]
    elif logits == "hilo8":
        decls += [("xp8", [H, 2 * T], F8), ("e1p8", [H, 2 * S], F8),
                  ("e2p8", [H, 2 * St], F8)]
    for nm, shape, ddt in decls:
        d[nm] = nc.dram_tensor(nm, shape, ddt, kind="ExternalInput")
    d["aw1"] = nc.dram_tensor("aw1", [T, S], F32, kind="ExternalOutput")
    d["aw2"] = nc.dram_tensor("aw2", [T, St], F32, kind="ExternalOutput")
    d["fusT"] = nc.dram_tensor("fusT", [H, T], F32, kind="ExternalOutput")

    with tile.TileContext(nc) as tc, contextlib.ExitStack() as ctx:
        P = ctx.enter_context(tc.tile_pool(name="persist", bufs=1))

        ident_t = P.tile([128, 128], F16, tag="ident")
        nc.gpsimd.dma_start(ident_t[:], d["ident"].ap()[:, :])

        def load_blocked(tag, dram, nblk, width, ddt, eng=None, nsplit=1):
            t = P.tile([128, nblk * width], ddt, tag=tag)
            eng = eng or nc.sync
            cw = width // nsplit
            for sp in range(nsplit):
                for b_ in range(nblk):
                    eng.dma_start(
                        t[:, b_ * width + sp * cw: b_ * width + (sp + 1) * cw],
                        dram.ap()[b_ * 128:(b_ + 1) * 128, sp * cw:(sp + 1) * cw])
            return t

        pending = []  # (a16 tile, it, nsb, a_t) transposes deferred one block

        def attention(it_range, nsb, xt_hi_t, xt_lo_t, et_hi_t, et_lo_t, a_t,
                      aw_dram, PSA, PST, WRK, STT, width, corr=None):
            """One encoder's logits+softmax+transpose. width = S or St.

            Transposes for tile `it` are emitted after the logits of `it+1`
            so the softmax chain (DVE/ACT) has a full logits-block of slack
            before the PE reaches the transpose instructions.
            """
            nhalf = width // 512

            def flush_pending():
                for a16p, itp, nsbp, a_tp in pending:
                    for sb in range(nsbp):
                        pt = PST.tile([128, 128], F16, tag="tr")
                        nc.tensor.transpose(
                            pt[:], a16p[:, sb * 128:(sb + 1) * 128], ident_t[:])
                        nc.vector.tensor_copy(
                            a_tp[:, sb * T + itp * 128: sb * T + itp * 128 + 128],
                            pt[:])
                pending.clear()

            # Tiles processed in pairs, pass-major: both tiles' (hi,hi) pass
            # runs before any (lo,*) pass, so the kernel head only waits on
            # the hi tensors' DMA; pass order matches the DMA issue order.
            if xt_lo_t is None:
                passes = ((xt_hi_t, et_hi_t),)
            else:
                passes = ((xt_hi_t, et_hi_t), (xt_lo_t, et_hi_t),
                          (xt_hi_t, et_lo_t))
            it_list = list(it_range)
            for it0 in range(0, len(it_list), 2):
                pair = it_list[it0:it0 + 2]
                ps = {it: [PSA.tile([128, 512], F32, tag="p",
                                    name=f"p{it % 2}{sh}")
                           for sh in range(nhalf)] for it in pair}
                if len(passes) == 1:
                    # single-pass logits: s-half-outer so the first groups
                    # only need the first half of e^T from DRAM
                    lt, rt = passes[0]
                    for sh in range(nhalf):
                        for it in pair:
                            for hb in range(HB):
                                nc.tensor.matmul(
                                    ps[it][sh][:],
                                    lt[:, hb * T + it * 128:
                                       hb * T + it * 128 + 128],
                                    rt[:, hb * width + sh * 512:
                                       hb * width + sh * 512 + 512],
                                    start=(hb == 0), stop=(hb == HB - 1))
                    if corr is not None:
                        # fp8 DoubleRow correction: one matmul per h-block
                        # computes Xhi@(E_lo*256) + (Xlo*256)@E_hi; combined
                        # into SBUF with a 1/256 rescale.
                        xp8_t, ep8_t = corr
                        psb = {}
                        for it in pair:
                            psb[it] = WRK.tile([128, width], F32, tag="psb",
                                               bufs=3, name=f"psb{it % 2}")
                            for sh in range(nhalf):
                                pcr = PSA.tile([128, 512], F32, tag="corr",
                                               bufs=2, name=f"pc{it % 2}{sh}")
                                for hb in range(HB):
                                    lhsT = xp8_t[
                                        :, hb * 2 * T:(hb + 1) * 2 * T
                                    ].rearrange("p (two t) -> p two t", two=2)[
                                        :, :, it * 128:(it + 1) * 128]
                                    rhs = ep8_t[
                                        :, hb * 2 * width:(hb + 1) * 2 * width
                                    ].rearrange("p (two s) -> p two s", two=2)[
                                        :, :, sh * 512:(sh + 1) * 512]
                                    nc.tensor.matmul(
                                        pcr[:], lhsT, rhs,
                                        start=(hb == 0), stop=(hb == HB - 1),
                                        perf_mode=mybir.MatmulPerfMode.DoubleRow)
                                corr_s = WRK.tile([128, 512], F32, tag="corr_s",
                                                  bufs=3, name=f"cs{it % 2}{sh}")
                                nc.scalar.activation(corr_s[:], pcr[:], AF.Copy,
                                                     scale=1.0 / 256.0)
                                nc.vector.tensor_add(
                                    psb[it][:, sh * 512:(sh + 1) * 512],
                                    ps[it][sh][:], corr_s[:])
                        for it in pair:
                            ps[it] = [psb[it][:, sh * 512:(sh + 1) * 512]
                                      for sh in range(nhalf)]
                else:
                    for pi, (lt, rt) in enumerate(passes):
                        order = [(sh, it, hb) for it in pair
                                 for hb in range(HB)
                                 for sh in range(nhalf)]
                        for sh, it, hb in order:
                            nc.tensor.matmul(
                                ps[it][sh][:],
                                lt[:, hb * T + it * 128:
                                   hb * T + it * 128 + 128],
                                rt[:, hb * width + sh * 512:
                                   hb * width + sh * 512 + 512],
                                start=(pi == 0 and hb == 0),
                                stop=(pi == len(passes) - 1
                                      and hb == HB - 1))
                flush_pending()
                for it in pair:
                    softmax_tile(it, ps[it], nsb, a_t, aw_dram, WRK, STT, width,
                                 nhalf)
            return flush_pending

        def softmax_tile(it, ps, nsb, a_t, aw_dram, WRK, STT, width, nhalf):
                nm = STT.tile([128, 1], F32, tag="nm")
                if nhalf == 1:
                    nc.vector.tensor_reduce(nm[:], ps[0][:], axis=AX.X,
                                            op=ALU.max, negate=True)
                else:
                    nm0 = STT.tile([128, 1], F32, tag="nm0")
                    nm1 = STT.tile([128, 1], F32, tag="nm1")
                    nc.vector.tensor_reduce(nm0[:], ps[0][:], axis=AX.X,
                                            op=ALU.max, negate=True)
                    nc.vector.tensor_reduce(nm1[:], ps[1][:], axis=AX.X,
                                            op=ALU.max, negate=True)
                    nc.vector.tensor_tensor(nm[:], nm0[:], nm1[:], op=ALU.min)
                a_f32 = WRK.tile([128, width], F32, tag="a_f32")
                ssum = STT.tile([128, 1], F32, tag="ssum")
                if nhalf == 1:
                    nc.scalar.activation(a_f32[:], ps[0][:], AF.Exp,
                                         bias=nm[:], scale=1.0, accum_out=ssum[:])
                else:
                    s0 = STT.tile([128, 1], F32, tag="s0")
                    s1 = STT.tile([128, 1], F32, tag="s1")
                    nc.scalar.activation(a_f32[:, 0:512], ps[0][:], AF.Exp,
                                         bias=nm[:], scale=1.0, accum_out=s0[:])
                    nc.scalar.activation(a_f32[:, 512:1024], ps[1][:], AF.Exp,
                                         bias=nm[:], scale=1.0, accum_out=s1[:])
                    nc.vector.tensor_add(ssum[:], s0[:], s1[:])
                rinv = STT.tile([128, 1], F32, tag="rinv")
                nc.vector.reciprocal(rinv[:], ssum[:])
                an = WRK.tile([128, width], F32, tag="an")
                nc.scalar.activation(an[:], a_f32[:], AF.Copy, scale=rinv[:])
                nc.sync.dma_start(aw_dram.ap()[it * 128:(it + 1) * 128, :], an[:])
                a16 = WRK.tile([128, width], F16, tag="a16", bufs=4)
                nc.vector.tensor_scalar_mul(a16[:], a_f32[:], rinv[:])
                pending.append((a16, it, nsb, a_t))

        def ctx_matmul(c_t, ef_t, a_t, nsb, PSA):
            """C^T[k,t] = sum_s E[s,k] * A^T[s,t]. t-halves share lhsT."""
            for kt in range(HB):
                pcs = [PSA.tile([128, 512], F32, tag="p", name=f"pc{tb}")
                       for tb in range(NT)]
                for sb in range(nsb):
                    lhsT = ef_t[:, sb * H + kt * 128: sb * H + kt * 128 + 128]
                    for tb in range(NT):
                        nc.tensor.matmul(
                            pcs[tb][:], lhsT,
                            a_t[:, sb * T + tb * 512: sb * T + tb * 512 + 512],
                            start=(sb == 0), stop=(sb == nsb - 1))
                for tb in range(NT):
                    nc.scalar.copy(
                        c_t[:, kt * T + tb * 512: kt * T + tb * 512 + 512],
                        pcs[tb][:])

        for _rep in range(reps):
            pending.clear()
            # critical-path loads first: t-tile 0 logits need all of xt_hi and
            # e1t_hi; xt_lo/e1t_lo are needed one matmul-group later.
            # interleave column-half chunks in the order the first two
            # tile-pairs consume them: xt-c0, e1t-h0, e1t-h1, xt-c1
            xt_hi_t = P.tile([128, HB * T], LG, tag="xt_hi")
            e1t_hi_t = P.tile([128, HB * S], LG, tag="e1t_hi")
            for t_, dram, wid, sp in ((xt_hi_t, d["xt_hi"], T, 0),
                                      (e1t_hi_t, d["e1t_hi"], S, 0),
                                      (e1t_hi_t, d["e1t_hi"], S, 1),
                                      (xt_hi_t, d["xt_hi"], T, 1)):
                for b_ in range(HB):
                    nc.sync.dma_start(
                        t_[:, b_ * wid + sp * 512: b_ * wid + sp * 512 + 512],
                        dram.ap()[b_ * 128:(b_ + 1) * 128,
                                  sp * 512:(sp + 1) * 512])
            if logits == "hilo":
                xt_lo_t = load_blocked("xt_lo", d["xt_lo"], HB, T, BF16, nc.sync)
                e1t_lo_t = load_blocked("e1t_lo", d["e1t_lo"], HB, S, BF16,
                                        nc.sync)
            else:
                xt_lo_t = e1t_lo_t = e2t_lo_t = None
            if logits == "hilo8":
                xp8_t = load_blocked("xp8", d["xp8"], HB, 2 * T, F8, nc.sync)
                e1p8_t = load_blocked("e1p8", d["e1p8"], HB, 2 * S, F8, nc.sync)
            else:
                xp8_t = e1p8_t = e2p8_t = None
            # non-critical loads behind the criticals on the same queue, so
            # they never compete with them for HBM bandwidth at the head
            e2t_hi_t = load_blocked("e2t_hi", d["e2t_hi"], HB, St, LG, nc.sync)
            if logits == "hilo":
                e2t_lo_t = load_blocked("e2t_lo", d["e2t_lo"], HB, St, BF16,
                                        nc.sync)
            if logits == "hilo8":
                e2p8_t = load_blocked("e2p8", d["e2p8"], HB, 2 * St, F8, nc.sync)
            e1f_t = load_blocked("e1f", d["e1f"], SB, H, F16, nc.sync)
            e2f_t = load_blocked("e2f", d["e2f"], S2B, H, F16, nc.sync)
            a1t_t = P.tile([128, SB * T], F16, tag="a1t")
            a2t_t = P.tile([128, S2B * T], F16, tag="a2t")

            with tc.tile_pool(name="psA", bufs=(4 if logits == "hilo8" else 6),
                              space="PSUM") as PSA, \
                 tc.tile_pool(name="psT", bufs=2, space="PSUM") as PST, \
                 tc.tile_pool(name="wrkA", bufs=2) as WRK, \
                 tc.tile_pool(name="stats", bufs=4) as STT:
                # sent attention, then template attention (fills PE while the
                # tail of sent softmax/transposes completes), then both C^T.
                attention(range(TB), SB, xt_hi_t, xt_lo_t, e1t_hi_t, e1t_lo_t,
                          a1t_t, d["aw1"], PSA, PST, WRK, STT, S,
                          corr=(xp8_t, e1p8_t) if xp8_t is not None else None)
                flush_last = attention(
                    range(TB), S2B, xt_hi_t, xt_lo_t, e2t_hi_t, e2t_lo_t,
                    a2t_t, d["aw2"], PSA, PST, WRK, STT, St,
                    corr=(xp8_t, e2p8_t) if xp8_t is not None else None)
                flush_last()
                c1t_t = P.tile([128, HB * T], F16, tag="e1t_hi")  # slot reuse
                ctx_matmul(c1t_t, e1f_t, a1t_t, SB, PSA)
                c2t_t = P.tile([128, HB * T], F16, tag="e1t_lo")  # slot reuse
                ctx_matmul(c2t_t, e2f_t, a2t_t, S2B, PSA)

            # fused gate + fusion stage (all f16 matmuls)
            xtf_t = load_blocked("xt_hi", d["xtf"], HB, T, F16)  # slot reuse
            with tc.tile_pool(name="psC", bufs=2, space="PSUM") as PSC, \
                 tc.tile_pool(name="wrkC", bufs=2) as WC:
                for ht in range(HB):
                    wgt = WC.tile([128, 24 * 128], F16, tag="wg")
                    nc.sync.dma_start(wgt[:], d["wg"].ap()[ht * 128:(ht + 1) * 128, :])
                    wsto = WC.tile([128, 3 * 8 * 128], F16, tag="wsto")
                    for j, wnm in enumerate(("ws", "wt", "wo")):
                        nc.sync.dma_start(
                            wsto[:, j * 1024:(j + 1) * 1024],
                            d[wnm].ap()[ht * 128:(ht + 1) * 128, :])
                    for tb in range(NT):
                        pg = PSC.tile([128, 512], F32, tag="g")
                        i = 0
                        for src in (c1t_t, c2t_t, xtf_t):
                            for kb in range(HB):
                                nc.tensor.matmul(
                                    pg[:], wgt[:, i * 128:(i + 1) * 128],
                                    src[:, kb * T + tb * 512: kb * T + tb * 512 + 512],
                                    start=(i == 0), stop=(i == 23))
                                i += 1
                        pf = []
                        for j, src in enumerate((c1t_t, c2t_t, xtf_t)):
                            pfj = PSC.tile([128, 512], F32, tag=f"f{j}")
                            for kb in range(HB):
                                nc.tensor.matmul(
                                    pfj[:],
                                    wsto[:, j * 1024 + kb * 128: j * 1024 + kb * 128 + 128],
                                    src[:, kb * T + tb * 512: kb * T + tb * 512 + 512],
                                    start=(kb == 0), stop=(kb == HB - 1))
                            pf.append(pfj)
                        g_s = WC.tile([128, 512], F32, tag="g_s")
                        nc.scalar.activation(g_s[:], pg[:], AF.Sigmoid)
                        f1_s = WC.tile([128, 512], F32, tag="f1_s")
                        nc.scalar.copy(f1_s[:], pf[0][:])
                        d_s = WC.tile([128, 512], F32, tag="d_s")
                        nc.vector.tensor_sub(d_s[:], pf[1][:], f1_s[:])
                        f13 = WC.tile([128, 512], F32, tag="f13")
                        nc.vector.tensor_add(f13[:], pf[2][:], f1_s[:])
                        t1 = WC.tile([128, 512], F32, tag="t1")
                        nc.vector.tensor_mul(t1[:], d_s[:], g_s[:])
                        t2 = WC.tile([128, 512], F32, tag="t2")
                        nc.vector.tensor_add(t2[:], t1[:], f13[:])
                        fo = WC.tile([128, 512], F32, tag="fo")
                        nc.scalar.activation(fo[:], t2[:], AF.Tanh)
                        nc.sync.dma_start(
                            d["fusT"].ap()[ht * 128:(ht + 1) * 128,
                                           tb * 512:(tb + 1) * 512], fo[:])

    nc.compile()
    return nc


def _get_nc(reps=1, bc="f16", logits="hilo"):
    key = (reps, bc, logits)
    if key not in _CACHE:
        _CACHE[key] = _build(reps, bc, logits)
    return _CACHE[key]


def _prep_weight(w_t, cb, ob):
    """Pack W' [C,O] f16 so the per-output-tile lhsT DMA is contiguous.

    result[ot*128+p, kb*128+oo] = W'[kb*128+p, ot*128+oo]
    """
    return np.ascontiguousarray(
        w_t.reshape(cb, 128, ob, 128).transpose(2, 1, 0, 3).reshape(ob * 128, cb * 128))


def _hi_lo(x):
    hi = x.astype(ml_dtypes.bfloat16)
    lo = (x - hi.astype(np.float32)).astype(ml_dtypes.bfloat16)
    return hi, lo


def kernel(output, sent, template, W_gate, W_sent, W_template, W_output,
           _reps=None, _trace=False):
    from concourse.bass_utils import run_bass_kernel_spmd

    reps = _reps if _reps is not None else int(os.environ.get("BENCH_REPS", "1"))
    bc = os.environ.get("BENCH_BC", "f16")
    logits = os.environ.get("BENCH_LOGITS", "hilo")
    nc = _get_nc(reps, bc, logits)

    f16 = np.float16 if bc == "f16" else ml_dtypes.bfloat16
    wg_p = _prep_weight(np.ascontiguousarray(W_gate.T).astype(f16), 24, 8)
    ws_p = _prep_weight(np.ascontiguousarray(W_sent.T).astype(f16), 8, 8)
    wt_p = _prep_weight(np.ascontiguousarray(W_template.T).astype(f16), 8, 8)
    wo_p = _prep_weight(np.ascontiguousarray(W_output.T).astype(f16), 8, 8)
    ident = np.eye(128, dtype=f16)

    in_maps = []
    for b in range(B):
        xt = np.ascontiguousarray(output[b].T)
        e1t = np.ascontiguousarray(sent[b].T)
        e2t = np.ascontiguousarray(template[b].T)
        m = {
            "e1f": sent[b].astype(f16), "e2f": template[b].astype(f16),
            "xtf": xt.astype(f16),
            "wg": wg_p, "ws": ws_p, "wt": wt_p, "wo": wo_p,
            "ident": ident,
        }
        if logits == "hilo":
            m["xt_hi"], m["xt_lo"] = _hi_lo(xt)
            m["e1t_hi"], m["e1t_lo"] = _hi_lo(e1t)
            m["e2t_hi"], m["e2t_lo"] = _hi_lo(e2t)
        elif logits == "hilo8":
            f8 = ml_dtypes.float8_e4m3
            for nm_, arr in (("xt", xt), ("e1t", e1t), ("e2t", e2t)):
                hi, lo = _hi_lo(arr)
                m[f"{nm_}_hi" if nm_ != "xt" else "xt_hi"] = hi
                hi_f = hi.astype(np.float32)
                lo_f = lo.astype(np.float32) * 256.0
                pk_w = np.stack([hi_f, lo_f], axis=1).astype(f8)     # lhsT pairs
                pk_i = np.stack([lo_f, hi_f], axis=1).astype(f8)     # rhs pairs
                if nm_ == "xt":
                    m["xp8"] = np.ascontiguousarray(
                        pk_w.reshape(arr.shape[0], -1))
                else:
                    m[f"{nm_[:2]}p8"] = np.ascontiguousarray(
                        pk_i.reshape(arr.shape[0], -1))
        else:
            m["xt_hi"], m["e1t_hi"], m["e2t_hi"] = (
                xt.astype(np.float32), e1t.astype(np.float32),
                e2t.astype(np.float32))
        in_maps.append(m)

    res = run_bass_kernel_spmd(nc, in_maps, list(range(B)), trace=_trace)
    kernel.last_results = res

    fusion = np.stack([np.ascontiguousarray(res.results[b]["fusT"].T)
                       for b in range(B)])
    sent_weight = np.stack([res.results[b]["aw1"] for b in range(B)])
    template_weight = np.stack([res.results[b]["aw2"] for b in range(B)])
    return fusion, sent_weight, template_weight


# revision 79
# speedup vs baseline: 1.1258x; 1.1258x over previous
"""TRN2 Bass kernel for the two-encoder attention module.

Per batch element b (8 of them, one per NeuronCore):
    P1 = X @ E1^T          (T,S)   attention logits vs `sent`
    A1 = softmax(P1)               -> output sent_weight
    C1 = A1 @ E1           (T,H)
    P2/A2/C2 vs `template` (St)
    gate = sigmoid(cat(C1, C2, X) @ W_gate^T)        (T,H)
    fusion = tanh((1-gate)*C1@Ws^T + gate*C2@Wt^T + X@Wo^T)

Everything on-device is computed in a feature-major ("transposed") layout so
that every matmul contraction dim lands on SBUF partitions with no on-device
input transposes:
    - logits: lhsT = X^T (hi/lo bf16 split, 3-pass for fp32-grade accuracy),
      rhs = E^T (hi/lo).  Softmax along the free dim.
    - A^T obtained with PE transpose-mode matmuls (f16).
    - C^T  = E^T(natural lhsT) @ A^T    [f16]
    - gate^T, F^T via host-pretransposed/prepacked f16 weights.
Host side handles batch sharding across 8 cores, input transposes/casts and
the final un-transpose of fusion.
"""
import contextlib
import os

import ml_dtypes
import numpy as np

B, T, S, St, H = 8, 1024, 1024, 512, 1024
TB = T // 128           # 8 t-tiles
HB = H // 128           # 8 h-blocks
SB = S // 128           # 8 s-blocks (sent)
S2B = St // 128         # 4 s-blocks (template)
NT = T // 512           # 2 t-halves for 512-wide rhs

_CACHE = {}


def _build(reps=1, bc="f16", logits="hilo"):
    import concourse.bacc as bacc
    import concourse.mybir as mybir
    import concourse.tile as tile

    dt = mybir.dt
    F32, F16, BF16, F8 = dt.float32, dt.float16, dt.bfloat16, dt.float8e4
    if bc == "bf16":
        F16 = BF16  # dtype for the value/gate/fusion stages
    LG = dt.float32r if logits == "f32r" else BF16
    AF = mybir.ActivationFunctionType
    ALU = mybir.AluOpType
    AX = mybir.AxisListType

    nc = bacc.Bacc("TRN2", target_bir_lowering=False, debug=False, num_devices=8)

    d = {}
    decls = [
        ("xt_hi", [H, T], LG),
        ("e1t_hi", [H, S], LG),
        ("e2t_hi", [H, St], LG),
        ("e1f", [S, H], F16), ("e2f", [St, H], F16), ("xtf", [H, T], F16),
        ("wg", [H, 3 * H], F16), ("ws", [H, H], F16),
        ("wt", [H, H], F16), ("wo", [H, H], F16),
        ("ident", [128, 128], F16),
    ]
    if logits == "hilo":
        decls += [("xt_lo", [H, T], BF16), ("e1t_lo", [H, S], BF16),
                  ("e2t_lo", [H, St], BF16)]
    elif logits == "hilo8":
        decls += [("xp8", [H, 2 * T], F8), ("e1p8", [H, 2 * S], F8),
                  ("e2p8", [H, 2 * St], F8)]
    for nm, shape, ddt in decls:
        d[nm] = nc.dram_tensor(nm, shape, ddt, kind="ExternalInput")
    d["aw1"] = nc.dram_tensor("aw1", [T, S], F32, kind="ExternalOutput")
    d["aw2"] = nc.dram_tensor("aw2", [T, St], F32, kind="ExternalOutput")
    d["fusT"] = nc.dram_tensor("fusT", [H, T], F32, kind="ExternalOutput")

    with tile.TileContext(nc) as tc, contextlib.ExitStack() as ctx:
        P = ctx.enter_context(tc.tile_pool(name="persist", bufs=1))

        ident_t = P.tile([128, 128], F16, tag="ident")
        nc.gpsimd.dma_start(ident_t[:], d["ident"].ap()[:, :])

        def load_blocked(tag, dram, nblk, width, ddt, eng=None, nsplit=1):
            t = P.tile([128, nblk * width], ddt, tag=tag)
            eng = eng or nc.sync
            cw = width // nsplit
            for sp in range(nsplit):
                for b_ in range(nblk):
                    eng.dma_start(
                        t[:, b_ * width + sp * cw: b_ * width + (sp + 1) * cw],
                        dram.ap()[b_ * 128:(b_ + 1) * 128, sp * cw:(sp + 1) * cw])
            return t

        pending = []  # (a16 tile, it, nsb, a_t) transposes deferred one block
        corr_jobs = []  # deferred fp8 correction jobs (hilo8 mode)

        def attention(it_range, nsb, xt_hi_t, xt_lo_t, et_hi_t, et_lo_t, a_t,
                      aw_dram, PSA, PST, WRK, STT, width, corr=None):
            """One encoder's logits+softmax+transpose. width = S or St.

            Transposes for tile `it` are emitted after the logits of `it+1`
            so the softmax chain (DVE/ACT) has a full logits-block of slack
            before the PE reaches the transpose instructions.
            """
            nhalf = width // 512

            def flush_pending():
                for a16p, itp, nsbp, a_tp in pending:
                    for sb in range(nsbp):
                        pt = PST.tile([128, 128], F16, tag="tr")
                        nc.tensor.transpose(
                            pt[:], a16p[:, sb * 128:(sb + 1) * 128], ident_t[:])
                        nc.vector.tensor_copy(
                            a_tp[:, sb * T + itp * 128: sb * T + itp * 128 + 128],
                            pt[:])
                pending.clear()

            def flush_corr(jobs):
                for (it, pms, nsbj, a_tj, awj, wj, nhj, xp8_t, ep8_t) in jobs:
                    psb = WRK.tile([128, wj], F32, tag="psb", bufs=3,
                                   name=f"psb{it % 2}")
                    for sh in range(nhj):
                        pcr = PSA.tile([128, 512], F32, tag="corr", bufs=2,
                                       name=f"pcr{it % 2}{sh}")
                        for hb in range(HB):
                            lhsT = xp8_t[
                                :, hb * 2 * T:(hb + 1) * 2 * T
                            ].rearrange("p (two t) -> p two t", two=2)[
                                :, :, it * 128:(it + 1) * 128]
                            rhs = ep8_t[
                                :, hb * 2 * wj:(hb + 1) * 2 * wj
                            ].rearrange("p (two s) -> p two s", two=2)[
                                :, :, sh * 512:(sh + 1) * 512]
                            nc.tensor.matmul(
                                pcr[:], lhsT, rhs,
                                start=(hb == 0), stop=(hb == HB - 1),
                                perf_mode=mybir.MatmulPerfMode.DoubleRow)
                        nc.vector.scalar_tensor_tensor(
                            psb[:, sh * 512:(sh + 1) * 512], pcr[:],
                            1.0 / 256.0, pms[sh][:],
                            op0=ALU.mult, op1=ALU.add)
                    softmax_tile(it, [psb[:, sh * 512:(sh + 1) * 512]
                                      for sh in range(nhj)],
                                 nsbj, a_tj, awj, WRK, STT, wj, nhj)

            # Tiles processed in pairs, pass-major: both tiles' (hi,hi) pass
            # runs before any (lo,*) pass, so the kernel head only waits on
            # the hi tensors' DMA; pass order matches the DMA issue order.
            if xt_lo_t is None:
                passes = ((xt_hi_t, et_hi_t),)
            else:
                passes = ((xt_hi_t, et_hi_t), (xt_lo_t, et_hi_t),
                          (xt_hi_t, et_lo_t))
            it_list = list(it_range)
            for it0 in range(0, len(it_list), 2):
                pair = it_list[it0:it0 + 2]
                ps = {it: [PSA.tile([128, 512], F32, tag="p",
                                    name=f"p{it % 2}{sh}")
                           for sh in range(nhalf)] for it in pair}
                if len(passes) == 1:
                    # single-pass logits: s-half-outer so the first groups
                    # only need the first half of e^T from DRAM
                    lt, rt = passes[0]
                    for sh in range(nhalf):
                        for it in pair:
                            for hb in range(HB):
                                nc.tensor.matmul(
                                    ps[it][sh][:],
                                    lt[:, hb * T + it * 128:
                                       hb * T + it * 128 + 128],
                                    rt[:, hb * width + sh * 512:
                                       hb * width + sh * 512 + 512],
                                    start=(hb == 0), stop=(hb == HB - 1))
                    if corr is not None:
                        # Copy main-pass psums to SBUF (frees the banks) and
                        # defer this pair's fp8 DoubleRow correction one pair,
                        # so it never stalls on the fp8 pair-tensor DMAs.
                        xp8_t, ep8_t = corr
                        for it in pair:
                            pms = []
                            for sh in range(nhalf):
                                pm = WRK.tile([128, 512], F32, tag="pm_s",
                                              bufs=8, name=f"pm{it % 2}{sh}")
                                nc.scalar.copy(pm[:], ps[it][sh][:])
                                pms.append(pm)
                            corr_jobs.append((it, pms, nsb, a_t, aw_dram,
                                              width, nhalf, xp8_t, ep8_t))
                else:
                    for pi, (lt, rt) in enumerate(passes):
                        order = [(sh, it, hb) for it in pair
                                 for hb in range(HB)
                                 for sh in range(nhalf)]
                        for sh, it, hb in order:
                            nc.tensor.matmul(
                                ps[it][sh][:],
                                lt[:, hb * T + it * 128:
                                   hb * T + it * 128 + 128],
                                rt[:, hb * width + sh * 512:
                                   hb * width + sh * 512 + 512],
                                start=(pi == 0 and hb == 0),
                                stop=(pi == len(passes) - 1
                                      and hb == HB - 1))
                flush_pending()
                if corr is not None:
                    if len(corr_jobs) > 2:
                        flush_corr(corr_jobs[:-2])
                        del corr_jobs[:-2]
                else:
                    for it in pair:
                        softmax_tile(it, ps[it], nsb, a_t, aw_dram, WRK, STT,
                                     width, nhalf)
            if corr is not None:
                flush_corr(corr_jobs)
                corr_jobs.clear()
            return flush_pending

        def softmax_tile(it, ps, nsb, a_t, aw_dram, WRK, STT, width, nhalf):
                nm = STT.tile([128, 1], F32, tag="nm")
                if nhalf == 1:
                    nc.vector.tensor_reduce(nm[:], ps[0][:], axis=AX.X,
                                            op=ALU.max, negate=True)
                else:
                    nm0 = STT.tile([128, 1], F32, tag="nm0")
                    nm1 = STT.tile([128, 1], F32, tag="nm1")
                    nc.vector.tensor_reduce(nm0[:], ps[0][:], axis=AX.X,
                                            op=ALU.max, negate=True)
                    nc.vector.tensor_reduce(nm1[:], ps[1][:], axis=AX.X,
                                            op=ALU.max, negate=True)
                    nc.vector.tensor_tensor(nm[:], nm0[:], nm1[:], op=ALU.min)
                a_f32 = WRK.tile([128, width], F32, tag="a_f32")
                ssum = STT.tile([128, 1], F32, tag="ssum")
                if nhalf == 1:
                    nc.scalar.activation(a_f32[:], ps[0][:], AF.Exp,
                                         bias=nm[:], scale=1.0, accum_out=ssum[:])
                else:
                    s0 = STT.tile([128, 1], F32, tag="s0")
                    s1 = STT.tile([128, 1], F32, tag="s1")
                    nc.scalar.activation(a_f32[:, 0:512], ps[0][:], AF.Exp,
                                         bias=nm[:], scale=1.0, accum_out=s0[:])
                    nc.scalar.activation(a_f32[:, 512:1024], ps[1][:], AF.Exp,
                                         bias=nm[:], scale=1.0, accum_out=s1[:])
                    nc.vector.tensor_add(ssum[:], s0[:], s1[:])
                rinv = STT.tile([128, 1], F32, tag="rinv")
                nc.vector.reciprocal(rinv[:], ssum[:])
                an = WRK.tile([128, width], F32, tag="an")
                nc.scalar.activation(an[:], a_f32[:], AF.Copy, scale=rinv[:])
                nc.sync.dma_start(aw_dram.ap()[it * 128:(it + 1) * 128, :], an[:])
                a16 = WRK.tile([128, width], F16, tag="a16", bufs=4)
                nc.vector.tensor_scalar_mul(a16[:], a_f32[:], rinv[:])
                pending.append((a16, it, nsb, a_t))

        def ctx_matmul(c_t, ef_t, a_t, nsb, PSA):
            """C^T[k,t] = sum_s E[s,k] * A^T[s,t]. t-halves share lhsT."""
            for kt in range(HB):
                pcs = [PSA.tile([128, 512], F32, tag="p", name=f"pc{tb}")
                       for tb in range(NT)]
                for sb in range(nsb):
                    lhsT = ef_t[:, sb * H + kt * 128: sb * H + kt * 128 + 128]
                    for tb in range(NT):
                        nc.tensor.matmul(
                            pcs[tb][:], lhsT,
                            a_t[:, sb * T + tb * 512: sb * T + tb * 512 + 512],
                            start=(sb == 0), stop=(sb == nsb - 1))
                for tb in range(NT):
                    nc.scalar.copy(
                        c_t[:, kt * T + tb * 512: kt * T + tb * 512 + 512],
                        pcs[tb][:])

        for _rep in range(reps):
            pending.clear()
            # critical-path loads first: t-tile 0 logits need all of xt_hi and
            # e1t_hi; xt_lo/e1t_lo are needed one matmul-group later.
            # interleave column-half chunks in the order the first two
            # tile-pairs consume them: xt-c0, e1t-h0, e1t-h1, xt-c1
            xt_hi_t = P.tile([128, HB * T], LG, tag="xt_hi")
            e1t_hi_t = P.tile([128, HB * S], LG, tag="e1t_hi")
            for t_, dram, wid, sp in ((xt_hi_t, d["xt_hi"], T, 0),
                                      (e1t_hi_t, d["e1t_hi"], S, 0),
                                      (e1t_hi_t, d["e1t_hi"], S, 1),
                                      (xt_hi_t, d["xt_hi"], T, 1)):
                for b_ in range(HB):
                    nc.sync.dma_start(
                        t_[:, b_ * wid + sp * 512: b_ * wid + sp * 512 + 512],
                        dram.ap()[b_ * 128:(b_ + 1) * 128,
                                  sp * 512:(sp + 1) * 512])
            if logits == "hilo":
                xt_lo_t = load_blocked("xt_lo", d["xt_lo"], HB, T, BF16, nc.sync)
                e1t_lo_t = load_blocked("e1t_lo", d["e1t_lo"], HB, S, BF16,
                                        nc.sync)
            else:
                xt_lo_t = e1t_lo_t = e2t_lo_t = None
            if logits == "hilo8":
                xp8_t = load_blocked("xp8", d["xp8"], HB, 2 * T, F8, nc.sync)
                e1p8_t = load_blocked("e1p8", d["e1p8"], HB, 2 * S, F8, nc.sync)
            else:
                xp8_t = e1p8_t = e2p8_t = None
            # non-critical loads behind the criticals on the same queue, so
            # they never compete with them for HBM bandwidth at the head
            e2t_hi_t = load_blocked("e2t_hi", d["e2t_hi"], HB, St, LG, nc.sync)
            if logits == "hilo":
                e2t_lo_t = load_blocked("e2t_lo", d["e2t_lo"], HB, St, BF16,
                                        nc.sync)
            if logits == "hilo8":
                e2p8_t = load_blocked("e2p8", d["e2p8"], HB, 2 * St, F8, nc.sync)
            e1f_t = load_blocked("e1f", d["e1f"], SB, H, F16, nc.sync)
            e2f_t = load_blocked("e2f", d["e2f"], S2B, H, F16, nc.sync)
            a1t_t = P.tile([128, SB * T], F16, tag="a1t")
            a2t_t = P.tile([128, S2B * T], F16, tag="a2t")

            with tc.tile_pool(name="psA", bufs=(4 if logits == "hilo8" else 6),
                              space="PSUM") as PSA, \
                 tc.tile_pool(name="psT", bufs=2, space="PSUM") as PST, \
                 tc.tile_pool(name="wrkA", bufs=2) as WRK, \
                 tc.tile_pool(name="stats", bufs=4) as STT:
                # sent attention, then template attention (fills PE while the
                # tail of sent softmax/transposes completes), then both C^T.
                attention(range(TB), SB, xt_hi_t, xt_lo_t, e1t_hi_t, e1t_lo_t,
                          a1t_t, d["aw1"], PSA, PST, WRK, STT, S,
                          corr=(xp8_t, e1p8_t) if xp8_t is not None else None)
                flush_last = attention(
                    range(TB), S2B, xt_hi_t, xt_lo_t, e2t_hi_t, e2t_lo_t,
                    a2t_t, d["aw2"], PSA, PST, WRK, STT, St,
                    corr=(xp8_t, e2p8_t) if xp8_t is not None else None)
                flush_last()
                c1t_t = P.tile([128, HB * T], F16, tag="e1t_hi")  # slot reuse
                ctx_matmul(c1t_t, e1f_t, a1t_t, SB, PSA)
                c2t_t = P.tile([128, HB * T], F16, tag="e1t_lo")  # slot reuse
                ctx_matmul(c2t_t, e2f_t, a2t_t, S2B, PSA)

            # fused gate + fusion stage (all f16 matmuls)
            xtf_t = load_blocked("xt_hi", d["xtf"], HB, T, F16)  # slot reuse
            with tc.tile_pool(name="psC", bufs=2, space="PSUM") as PSC, \
                 tc.tile_pool(name="wrkC", bufs=2) as WC:
                for ht in range(HB):
                    wgt = WC.tile([128, 24 * 128], F16, tag="wg")
                    nc.sync.dma_start(wgt[:], d["wg"].ap()[ht * 128:(ht + 1) * 128, :])
                    wsto = WC.tile([128, 3 * 8 * 128], F16, tag="wsto")
                    for j, wnm in enumerate(("ws", "wt", "wo")):
                        nc.sync.dma_start(
                            wsto[:, j * 1024:(j + 1) * 1024],
                            d[wnm].ap()[ht * 128:(ht + 1) * 128, :])
                    for tb in range(NT):
                        pg = PSC.tile([128, 512], F32, tag="g")
                        i = 0
                        for src in (c1t_t, c2t_t, xtf_t):
                            for kb in range(HB):
                                nc.tensor.matmul(
                                    pg[:], wgt[:, i * 128:(i + 1) * 128],
                                    src[:, kb * T + tb * 512: kb * T + tb * 512 + 512],
                                    start=(i == 0), stop=(i == 23))
                                i += 1
                        pf = []
                        for j, src in enumerate((c1t_t, c2t_t, xtf_t)):
                            pfj = PSC.tile([128, 512], F32, tag=f"f{j}")
                            for kb in range(HB):
                                nc.tensor.matmul(
                                    pfj[:],
                                    wsto[:, j * 1024 + kb * 128: j * 1024 + kb * 128 + 128],
                                    src[:, kb * T + tb * 512: kb * T + tb * 512 + 512],
                                    start=(kb == 0), stop=(kb == HB - 1))
                            pf.append(pfj)
                        g_s = WC.tile([128, 512], F32, tag="g_s")
                        nc.scalar.activation(g_s[:], pg[:], AF.Sigmoid)
                        f1_s = WC.tile([128, 512], F32, tag="f1_s")
                        nc.scalar.copy(f1_s[:], pf[0][:])
                        d_s = WC.tile([128, 512], F32, tag="d_s")
                        nc.vector.tensor_sub(d_s[:], pf[1][:], f1_s[:])
                        f13 = WC.tile([128, 512], F32, tag="f13")
                        nc.vector.tensor_add(f13[:], pf[2][:], f1_s[:])
                        t1 = WC.tile([128, 512], F32, tag="t1")
                        nc.vector.tensor_mul(t1[:], d_s[:], g_s[:])
                        t2 = WC.tile([128, 512], F32, tag="t2")
                        nc.vector.tensor_add(t2[:], t1[:], f13[:])
                        fo = WC.tile([128, 512], F32, tag="fo")
                        nc.scalar.activation(fo[:], t2[:], AF.Tanh)
                        nc.sync.dma_start(
                            d["fusT"].ap()[ht * 128:(ht + 1) * 128,
                                           tb * 512:(tb + 1) * 512], fo[:])

    nc.compile()
    return nc


def _get_nc(reps=1, bc="f16", logits="hilo"):
    key = (reps, bc, logits)
    if key not in _CACHE:
        _CACHE[key] = _build(reps, bc, logits)
    return _CACHE[key]


def _prep_weight(w_t, cb, ob):
    """Pack W' [C,O] f16 so the per-output-tile lhsT DMA is contiguous.

    result[ot*128+p, kb*128+oo] = W'[kb*128+p, ot*128+oo]
    """
    return np.ascontiguousarray(
        w_t.reshape(cb, 128, ob, 128).transpose(2, 1, 0, 3).reshape(ob * 128, cb * 128))


def _hi_lo(x):
    hi = x.astype(ml_dtypes.bfloat16)
    lo = (x - hi.astype(np.float32)).astype(ml_dtypes.bfloat16)
    return hi, lo


def kernel(output, sent, template, W_gate, W_sent, W_template, W_output,
           _reps=None, _trace=False):
    from concourse.bass_utils import run_bass_kernel_spmd

    reps = _reps if _reps is not None else int(os.environ.get("BENCH_REPS", "1"))
    bc = os.environ.get("BENCH_BC", "f16")
    logits = os.environ.get("BENCH_LOGITS", "hilo")
    nc = _get_nc(reps, bc, logits)

    f16 = np.float16 if bc == "f16" else ml_dtypes.bfloat16
    wg_p = _prep_weight(np.ascontiguousarray(W_gate.T).astype(f16), 24, 8)
    ws_p = _prep_weight(np.ascontiguousarray(W_sent.T).astype(f16), 8, 8)
    wt_p = _prep_weight(np.ascontiguousarray(W_template.T).astype(f16), 8, 8)
    wo_p = _prep_weight(np.ascontiguousarray(W_output.T).astype(f16), 8, 8)
    ident = np.eye(128, dtype=f16)

    in_maps = []
    for b in range(B):
        xt = np.ascontiguousarray(output[b].T)
        e1t = np.ascontiguousarray(sent[b].T)
        e2t = np.ascontiguousarray(template[b].T)
        m = {
            "e1f": sent[b].astype(f16), "e2f": template[b].astype(f16),
            "xtf": xt.astype(f16),
            "wg": wg_p, "ws": ws_p, "wt": wt_p, "wo": wo_p,
            "ident": ident,
        }
        if logits == "hilo":
            m["xt_hi"], m["xt_lo"] = _hi_lo(xt)
            m["e1t_hi"], m["e1t_lo"] = _hi_lo(e1t)
            m["e2t_hi"], m["e2t_lo"] = _hi_lo(e2t)
        elif logits == "hilo8":
            f8 = ml_dtypes.float8_e4m3
            for nm_, arr in (("xt", xt), ("e1t", e1t), ("e2t", e2t)):
                hi, lo = _hi_lo(arr)
                m[f"{nm_}_hi" if nm_ != "xt" else "xt_hi"] = hi
                hi_f = hi.astype(np.float32)
                lo_f = lo.astype(np.float32) * 256.0
                pk_w = np.stack([hi_f, lo_f], axis=1).astype(f8)     # lhsT pairs
                pk_i = np.stack([lo_f, hi_f], axis=1).astype(f8)     # rhs pairs
                if nm_ == "xt":
                    m["xp8"] = np.ascontiguousarray(
                        pk_w.reshape(arr.shape[0], -1))
                else:
                    m[f"{nm_[:2]}p8"] = np.ascontiguousarray(
                        pk_i.reshape(arr.shape[0], -1))
        else:
            m["xt_hi"], m["e1t_hi"], m["e2t_hi"] = (
                xt.astype(np.float32), e1t.astype(np.float32),
                e2t.astype(np.float32))
        in_maps.append(m)

    res = run_bass_kernel_spmd(nc, in_maps, list(range(B)), trace=_trace)
    kernel.last_results = res

    fusion = np.stack([np.ascontiguousarray(res.results[b]["fusT"].T)
                       for b in range(B)])
    sent_weight = np.stack([res.results[b]["aw1"] for b in range(B)])
    template_weight = np.stack([res.results[b]["aw2"] for b in range(B)])
    return fusion, sent_weight, template_weight
